# revision 21
# baseline (speedup 1.0000x reference)
"""Causal attention (no 1/sqrt(d) scaling), B=8, S=2048, D=64, fp32 in/out.

Sharding: data-parallel over batch - one batch element per NeuronCore (8 cores).

Per-core algorithm (S=2048, D=64), v12:
  - All matmuls are bf16 (fp32r runs 2 cycles/row on TRN2's PE regardless of
    p-state; bf16 runs 1 and can ramp). bf16 QK costs rel-err ~7e-3 vs the
    2e-2 gate.
  - Host packs kT and qT bf16 into one [64, 4096] tensor, segments laid in
    consumption order so three sequential DMAs feed the chunks just in time.
  - Scores computed transposed per (q-chunk c, k-block j) into single-bank
    PSUM strips [128, 512], trimmed to the causal column range
    col_lo = max(0, 128j - 512c).  ScalarE exps exactly the causal columns
    (17408 activate-columns, the per-core floor) from PSUM into per-block
    bf16 SBUF tiles; DVE applies one shared [128,128] lower-tri mask on the
    16 diagonal blocks.
  - PV per q-block i: [128, 66] PSUM accumulates matmul(lhsT=exp block j,
    rhs=vx block j) over j<=i; vx col 64 = ones -> softmax denominator.
    DVE normalizes (fast reciprocal + scale) into a staging tile; outputs
    stream per chunk.
  - Single-bank strips (4 rotating) free two PSUM banks that hold
    pre-accumulation chains for the last two q-blocks (14, 15): their j<=13
    matmuls run while the final exps are still in flight, so after the last
    exp only two matmuls + a 33KB DMA remain (block 15 ships raw with its
    denominator; the host divides).
"""

import numpy as np

S = 2048
D = 64
B = 8
P = 128
CH = 512            # q-chunk width
NBLK = S // P       # 16 k-blocks
W = 66              # v | ones | pad

_CACHED = {}


def _build():
    import concourse.bass as bass
    import concourse.bacc as bacc
    import concourse.mybir as mybir
    import concourse.tile as tile

    f32 = mybir.dt.float32
    bf16 = mybir.dt.bfloat16
    Exp = mybir.ActivationFunctionType.Exp

    nc = bacc.Bacc("TRN2", target_bir_lowering=False, debug=False,
                   enable_asserts=False, num_devices=B)

    kq_d = nc.dram_tensor("kq", (64, 2 * S), bf16, kind="ExternalInput")
    vx_d = nc.dram_tensor("vx", (P, NBLK * W), bf16, kind="ExternalInput")
    mask_d = nc.dram_tensor("mask", (P, P), bf16, kind="ExternalInput")
    out_d = nc.dram_tensor("out", (P, NBLK * D), f32, kind="ExternalOutput")
    out15_d = nc.dram_tensor("out15", (P, W), f32, kind="ExternalOutput")

    with tile.TileContext(nc) as tc:
        with (
            tc.tile_pool(name="const", bufs=1) as cpool,
            tc.tile_pool(name="exps", bufs=40) as epool,
            tc.tile_pool(name="small", bufs=4) as spool,
            tc.tile_pool(name="spsum", bufs=4, space=bass.MemorySpace.PSUM) as sppool,
            tc.tile_pool(name="opsum", bufs=2, space=bass.MemorySpace.PSUM) as oppool,
            tc.tile_pool(name="prepsum", bufs=1, space=bass.MemorySpace.PSUM) as prepool,
        ):
            kq_s = cpool.tile([64, 2 * S], bf16, tag="kq", name="kq_s")
            vx_s = cpool.tile([P, NBLK * W], bf16, tag="vx", name="vx_s")
            mask_s = cpool.tile([P, P], bf16, tag="mask", name="mask_s")
            ostage = cpool.tile([P, NBLK * D], f32, tag="ostage", name="ostage_s")
            o15s = cpool.tile([P, W], f32, tag="o15s", name="o15s")
            scr_in = cpool.tile([P, 1], f32, tag="scr_in", name="scr_in")
            scr_out = cpool.tile([P, 1], f32, tag="scr_out", name="scr_out")

            # Input DMAs in consumption order; mask/vx issue on other engine
            # queues so their descriptors overlap the kq stream.
            nc.sync.dma_start(kq_s[:, 0:2 * CH], kq_d.ap()[:, 0:2 * CH])
            nc.scalar.dma_start(mask_s[:], mask_d.ap()[:])
            nc.gpsimd.dma_start(vx_s[:], vx_d.ap()[:])

            # Warm the Exp activation table during the DMA lead-in.
            nc.gpsimd.memset(scr_in[:], 0.0)
            nc.scalar.activation(scr_out[:], scr_in[:], Exp)
            nc.sync.dma_start(kq_s[:, 2 * CH:6 * CH], kq_d.ap()[:, 2 * CH:6 * CH])
            nc.sync.dma_start(kq_s[:, 6 * CH:2 * S], kq_d.ap()[:, 6 * CH:2 * S])

            # Packed kq column layout: 3 windows in consumption order, each
            # [q | k] for x-ranges [0,512), [512,1536), [1536,2048).
            _w0 = [0, 1024, 3072]
            _wx = [0, 512, 1536]
            _wn = [512, 1024, 512]

            def _seg(x):
                return 0 if x < 512 else 1 if x < 1536 else 2

            def pq(x):
                s = _seg(x)
                return _w0[s] + (x - _wx[s])

            def pk(x):
                s = _seg(x)
                return _w0[s] + _wn[s] + (x - _wx[s])

            eb = {}

            def qk_block(c, j):
                """Scores + exp (+ diag mask) for k-block j vs q-chunk c."""
                lo = max(0, P * j - CH * c)
                sp = sppool.tile([P, CH], f32, tag="scores", name="scores")
                kc = pk(j * P)
                qc = pq(c * CH + lo)
                nc.tensor.matmul(
                    sp[:, lo:CH], kq_s[:, kc:kc + P], kq_s[:, qc:qc + CH - lo],
                    start=True, stop=True)
                ebt = epool.tile([P, CH], bf16, tag="eb", name="eb")
                eb[(c, j)] = ebt
                nc.scalar.activation(ebt[:, lo:], sp[:, lo:], Exp)
                ii = j - 4 * c
                if 0 <= ii < 4:
                    col = P * ii
                    nc.vector.tensor_mul(
                        ebt[:, col:col + P], ebt[:, col:col + P], mask_s[:])

            def pv_mms(op, c, ii, js, start, stop):
                for j in js:
                    nc.tensor.matmul(
                        op[:], eb[(c, j)][:, ii * P:(ii + 1) * P],
                        vx_s[:, j * W:(j + 1) * W],
                        start=(start and j == js[0]),
                        stop=(stop and j == js[-1]))

            def pv_norm(op, i):
                rc = spool.tile([P, 1], f32, tag="recip", name="recip")
                nc.vector.reciprocal_approx_fast(rc[:], op[:, 64:65])
                nc.vector.tensor_scalar_mul(
                    ostage[:, i * D:(i + 1) * D], op[:, 0:D], rc[:])

            def pv_block(c, ii):
                op = oppool.tile([P, W], f32, tag="outp", name="outp")
                pv_mms(op, c, ii, list(range(4 * c + ii + 1)), True, True)
                pv_norm(op, 4 * c + ii)

            # Chunks 0-2: hoist the next chunk's first 4 k-blocks before PV
            # so the strip pipeline keeps feeding ScalarE while PV drains.
            for j in range(4):
                qk_block(0, j)
            for c in range(3):
                for j in range(4):
                    qk_block(c + 1, j)
                for ii in range(4):
                    pv_block(c, ii)
                nc.sync.dma_start(
                    out_d.ap()[:, 4 * c * D:(4 * c + 4) * D],
                    ostage[:, 4 * c * D:(4 * c + 4) * D])
                hi = 4 * (c + 1) + 4 if c < 2 else 10
                for j in range(4, hi):
                    qk_block(c + 1, j)

            # Chunk 3 close: pre-accumulate PV(14)/PV(15) over j<=13 in held
            # PSUM banks while the final exps run; after the last exp only
            # two matmuls + a raw 33KB DMA remain (host divides block 15).
            pre15 = prepool.tile([P, W], f32, tag="pre15", name="pre15")
            pre14 = prepool.tile([P, W], f32, tag="pre14", name="pre14")
            pv_mms(pre15, 3, 3, list(range(10)), True, False)
            pv_mms(pre14, 3, 2, list(range(10)), True, False)
            for j in (10, 11):
                qk_block(3, j)
            pv_mms(pre15, 3, 3, [10, 11], False, False)
            pv_mms(pre14, 3, 2, [10, 11], False, False)
            for j in (12, 13):
                qk_block(3, j)
            pv_mms(pre15, 3, 3, [12, 13], False, False)
            pv_mms(pre14, 3, 2, [12, 13], False, False)
            pv_block(3, 0)
            pv_block(3, 1)
            qk_block(3, 14)
            pv_mms(pre14, 3, 2, [14], False, True)
            pv_norm(pre14, 14)
            qk_block(3, 15)
            nc.sync.dma_start(
                out_d.ap()[:, 12 * D:15 * D], ostage[:, 12 * D:15 * D])
            pv_mms(pre15, 3, 3, [14, 15], False, True)
            nc.scalar.copy(o15s[:], pre15[:])
            nc.sync.dma_start(out15_d.ap()[:], o15s[:])

    nc.compile()
    return nc


def get_nc():
    if "nc" not in _CACHED:
        _CACHED["nc"] = _build()
    return _CACHED["nc"]


def make_in_maps(q, k, v):
    import ml_dtypes
    bf16 = ml_dtypes.bfloat16

    q = np.asarray(q, dtype=np.float32)
    k = np.asarray(k, dtype=np.float32)
    v = np.asarray(v, dtype=np.float32)

    kl = np.arange(P)[:, None]
    ql = np.arange(P)[None, :]
    mask = (ql >= kl).astype(bf16)

    in_maps = []
    for b in range(B):
        vx = np.zeros((NBLK, P, W), dtype=bf16)
        vx[:, :, :D] = v[b].reshape(NBLK, P, D).astype(bf16)
        vx[:, :, D] = bf16(1.0)
        vx = np.ascontiguousarray(
            vx.transpose(1, 0, 2)).reshape(P, NBLK * W)
        kT = k[b].T.astype(bf16)
        qh = q[b].T.astype(bf16)
        kq = np.concatenate([
            qh[:, 0:512], kT[:, 0:512],
            qh[:, 512:1536], kT[:, 512:1536],
            qh[:, 1536:2048], kT[:, 1536:2048],
        ], axis=1)
        in_maps.append({
            "kq": np.ascontiguousarray(kq),
            "vx": vx,
            "mask": mask,
        })
    return in_maps


def kernel(q, k, v):
    from concourse.bass_utils import run_bass_kernel_spmd

    nc = get_nc()
    in_maps = make_in_maps(q, k, v)
    res = run_bass_kernel_spmd(nc, in_maps, core_ids=list(range(B)))
    _CACHED["last_results"] = res
    outs = []
    for b in range(B):
        o = res.results[b]["out"].reshape(P, NBLK, D).copy()
        o15 = res.results[b]["out15"]
        o[:, 15, :] = o15[:, 0:D] / o15[:, D:D + 1]
        outs.append(o.transpose(1, 0, 2).reshape(S, D))
    return np.stack(outs, axis=0).astype(np.float32)


# revision 22
# speedup vs baseline: 1.0445x; 1.0445x over previous
"""Causal attention (no 1/sqrt(d) scaling), B=8, S=2048, D=64, fp32 in/out.

Sharding: data-parallel over batch - one batch element per NeuronCore (8 cores).

Per-core algorithm (S=2048, D=64), v2 (bf16 QK + causal trim + p-state aware):
  - All matmuls are bf16: TRN2's PE clock ramps 1.2 -> 2.4 GHz after ~3.5us of
    sustained bf16 matmul activity (fp32r matmuls never benefit: measured
    427ns/512-row at any p-state, while ramped bf16 runs 216ns/512-row).
    bf16 QK costs rel-err ~7e-3 (vs 1.8e-3 fp32) - well under the 2e-2 gate.
  - Host packs kT and qT into one [128, 2048] bf16 tensor (partitions 0-63 =
    kT d-major, 64-127 = qT d-major): one DMA stream at full 128-partition
    width, half the bytes of the old fp32 layout.
  - Scores computed transposed per (q-chunk c, k-block j): sT[k, q] =
    kq[0:64, j-block].T @ kq[64:128, q-chunk], trimmed to the causal column
    range (block granularity): col_lo = max(0, 128j - 512c). Strips are
    [128, 1024] PSUM pairs (2 k-blocks side by side).
  - Exp on ScalarE reads only the causal columns: 17408 activate-columns
    total (the hard floor; ScalarE is the bottleneck engine at ~19.5us).
  - Causal masking: one [128,128] lower-tri bf16 mask, tensor_mul on DVE over
    each of the 16 diagonal blocks after exp.
  - PV per q-block i: out[128, 66] PSUM accumulates matmul(lhsT=exp block,
    rhs=vx block) over j<=i; vx col 64 = ones -> softmax denominator.
  - Schedule interleaves chunk c's PV with chunk c+1's first QK pairs so
    ScalarE never idles at chunk boundaries.
  - Normalize: reciprocal_approx_fast + tensor_scalar_mul on DVE; one output
    DMA per chunk. Host un-permutes [128, 16*64] back to [2048, 64].
"""

import numpy as np

S = 2048
D = 64
B = 8
P = 128
CH = 512            # q-chunk width
NBLK = S // P       # 16 k-blocks
W = 66              # v | ones | pad

CORR = False        # q-split correction matmuls (2x QK rows, better accuracy)

_CACHED = {}


def _build():
    import concourse.bass as bass
    import concourse.bacc as bacc
    import concourse.mybir as mybir
    import concourse.tile as tile

    f32 = mybir.dt.float32
    bf16 = mybir.dt.bfloat16
    Exp = mybir.ActivationFunctionType.Exp

    nc = bacc.Bacc("TRN2", target_bir_lowering=False, debug=False,
                   enable_asserts=False, num_devices=B)

    assert not CORR, "CORR not supported in the [2,3,1,0] packed layout"
    kq_d = nc.dram_tensor("kq", (64, 2 * S), bf16, kind="ExternalInput")
    vx_d = nc.dram_tensor("vx", (P, NBLK * W), bf16, kind="ExternalInput")
    mask_d = nc.dram_tensor("mask", (P, P), bf16, kind="ExternalInput")
    out_d = nc.dram_tensor("out", (P, NBLK * D), f32, kind="ExternalOutput")
    out15_d = nc.dram_tensor("out15", (P, W), f32, kind="ExternalOutput")

    with tile.TileContext(nc) as tc:
        with (
            tc.tile_pool(name="const", bufs=1) as cpool,
            tc.tile_pool(name="exps", bufs=20) as epool,
            tc.tile_pool(name="small", bufs=4) as spool,
            tc.tile_pool(name="spsum", bufs=3, space=bass.MemorySpace.PSUM) as sppool,
            tc.tile_pool(name="opsum", bufs=2, space=bass.MemorySpace.PSUM) as oppool,
        ):
            kq_s = cpool.tile([64, 2 * S], bf16, tag="kq", name="kq_s")
            vx_s = cpool.tile([P, NBLK * W], bf16, tag="vx", name="vx_s")
            mask_s = cpool.tile([P, P], bf16, tag="mask", name="mask_s")
            ostage = cpool.tile([P, NBLK * D], f32, tag="ostage", name="ostage_s")
            scr_in = cpool.tile([P, 1], f32, tag="scr_in", name="scr_in")
            scr_out = cpool.tile([P, 1], f32, tag="scr_out", name="scr_out")

            # Input DMAs in consumption order (chunk 2 first); mask/vx on
            # other engine queues so their issue overlaps.
            nc.sync.dma_start(kq_s[:, 0:2 * CH], kq_d.ap()[:, 0:2 * CH])
            nc.scalar.dma_start(mask_s[:], mask_d.ap()[:])
            nc.gpsimd.dma_start(vx_s[:], vx_d.ap()[:])

            # Warm the Exp activation table during the DMA lead-in.
            nc.gpsimd.memset(scr_in[:], 0.0)
            nc.scalar.activation(scr_out[:], scr_in[:], Exp)
            nc.sync.dma_start(kq_s[:, 2 * CH:6 * CH], kq_d.ap()[:, 2 * CH:6 * CH])
            nc.sync.dma_start(kq_s[:, 6 * CH:8 * CH], kq_d.ap()[:, 6 * CH:8 * CH])

            # Packed kq column layout: 3 windows in consumption order, each
            # [q | k] for x-ranges [0,512), [512,1536), [1536,2048).
            _w0 = [0, 1024, 3072]
            _wx = [0, 512, 1536]
            _wn = [512, 1024, 512]

            def _seg(x):
                return 0 if x < 512 else 1 if x < 1536 else 2

            def pq_hi(x):
                s = _seg(x)
                return _w0[s] + (x - _wx[s])

            def pk(x):
                s = _seg(x)
                return _w0[s] + _wn[s] + (x - _wx[s])

            eb = {}

            def qk_pair(c, p):
                """Scores + exp (+ diag mask) for k-blocks (2p, 2p+1) vs chunk c."""
                j0, j1 = 2 * p, 2 * p + 1
                sp = sppool.tile([P, 2 * CH], f32, tag="scores", name="scores")
                for t, j in enumerate((j0, j1)):
                    lo = max(0, P * j - CH * c)
                    kc = pk(j * P)
                    qh = pq_hi(c * CH + lo)
                    nc.tensor.matmul(
                        sp[:, t * CH + lo:(t + 1) * CH],
                        kq_s[:, kc:kc + P],
                        kq_s[:, qh:qh + CH - lo],
                        start=True, stop=True)
                ebt = epool.tile([P, 2 * CH], bf16, tag="eb", name="eb")
                eb[(c, p)] = ebt
                if j1 < 4 * c:
                    nc.scalar.activation(ebt[:], sp[:], Exp)
                else:
                    for t, j in enumerate((j0, j1)):
                        lo = max(0, P * j - CH * c)
                        nc.scalar.activation(
                            ebt[:, t * CH + lo:(t + 1) * CH],
                            sp[:, t * CH + lo:(t + 1) * CH], Exp)
                for t, j in enumerate((j0, j1)):
                    ii = j - 4 * c
                    if 0 <= ii < 4:
                        col = t * CH + P * ii
                        nc.vector.tensor_mul(
                            ebt[:, col:col + P], ebt[:, col:col + P], mask_s[:])

            def pv_block(c, ii, raw=False):
                """PV accumulation + normalize for q-block i = 4c + ii."""
                i = 4 * c + ii
                op = oppool.tile([P, W], f32, tag="outp", name="outp")
                for j in range(i + 1):
                    ebt = eb[(c, j // 2)]
                    col = (j % 2) * CH + ii * P
                    nc.tensor.matmul(
                        op[:], ebt[:, col:col + P], vx_s[:, j * W:(j + 1) * W],
                        start=(j == 0), stop=(j == i))
                if raw:
                    return op
                rc = spool.tile([P, 1], f32, tag="recip", name="recip")
                nc.vector.reciprocal_approx_fast(rc[:], op[:, 64:65])
                nc.vector.tensor_scalar_mul(
                    ostage[:, i * D:(i + 1) * D], op[:, 0:D], rc[:])

            # Chunk order [0,1,2,3]; chunk 3's closing pairs run [.,4,6,7,5]
            # so the PV blocks gated on pairs 6/7 overlap the last full
            # pair's exp, shrinking the post-exp drain.
            for c in range(4):
                if c == 0:
                    qk_pair(0, 0)
                    qk_pair(0, 1)
                if c < 3:
                    qk_pair(c + 1, 0)
                    qk_pair(c + 1, 1)
                    for ii in range(4):
                        pv_block(c, ii)
                    nc.sync.dma_start(
                        out_d.ap()[:, 4 * c * D:(4 * c + 4) * D],
                        ostage[:, 4 * c * D:(4 * c + 4) * D])
                    for p in range(2, 2 * (c + 1) + 2):
                        qk_pair(c + 1, p)
                else:
                    for ii in range(3):
                        pv_block(3, ii)
                    nc.sync.dma_start(
                        out_d.ap()[:, 12 * D:15 * D], ostage[:, 12 * D:15 * D])
                    # Final block: ship the raw PSUM accumulator (with its
                    # denominator column) and normalize on the host - the
                    # last DMA then starts right after the last PV matmul.
                    op15 = pv_block(3, 3, raw=True)
                    o15s = cpool.tile([P, W], f32, tag="o15s", name="o15s")
                    nc.scalar.copy(o15s[:], op15[:])
                    nc.sync.dma_start(out15_d.ap()[:], o15s[:])

    nc.compile()
    return nc


def get_nc():
    if "nc" not in _CACHED:
        _CACHED["nc"] = _build()
    return _CACHED["nc"]


def make_in_maps(q, k, v):
    import ml_dtypes
    bf16 = ml_dtypes.bfloat16

    q = np.asarray(q, dtype=np.float32)
    k = np.asarray(k, dtype=np.float32)
    v = np.asarray(v, dtype=np.float32)

    kl = np.arange(P)[:, None]
    ql = np.arange(P)[None, :]
    mask = (ql >= kl).astype(bf16)

    in_maps = []
    for b in range(B):
        vx = np.zeros((NBLK, P, W), dtype=bf16)
        vx[:, :, :D] = v[b].reshape(NBLK, P, D).astype(bf16)
        vx[:, :, D] = bf16(1.0)
        vx = np.ascontiguousarray(
            vx.transpose(1, 0, 2)).reshape(P, NBLK * W)
        kT = k[b].T.astype(bf16)
        qh = q[b].T.astype(bf16)
        kq = np.concatenate([
            qh[:, 0:512], kT[:, 0:512],
            qh[:, 512:1536], kT[:, 512:1536],
            qh[:, 1536:2048], kT[:, 1536:2048],
        ], axis=1)
        in_maps.append({
            "kq": np.ascontiguousarray(kq),
            "vx": vx,
            "mask": mask,
        })
    return in_maps


def kernel(q, k, v):
    from concourse.bass_utils import run_bass_kernel_spmd

    nc = get_nc()
    in_maps = make_in_maps(q, k, v)
    res = run_bass_kernel_spmd(nc, in_maps, core_ids=list(range(B)))
    _CACHED["last_results"] = res
    outs = []
    for b in range(B):
        o = res.results[b]["out"].reshape(P, NBLK, D).copy()
        o15 = res.results[b]["out15"]
        o[:, 15, :] = o15[:, 0:D] / o15[:, D:D + 1]
        outs.append(o.transpose(1, 0, 2).reshape(S, D))
    return np.stack(outs, axis=0).astype(np.float32)


# revision 24
# speedup vs baseline: 1.0653x; 1.0200x over previous
"""Causal attention (no 1/sqrt(d) scaling), B=8, S=2048, D=64, fp32 in/out.

Sharding: data-parallel over batch - one batch element per NeuronCore (8 cores).

Per-core algorithm (S=2048, D=64), v2 (bf16 QK + causal trim + p-state aware):
  - All matmuls are bf16: TRN2's PE clock ramps 1.2 -> 2.4 GHz after ~3.5us of
    sustained bf16 matmul activity (fp32r matmuls never benefit: measured
    427ns/512-row at any p-state, while ramped bf16 runs 216ns/512-row).
    bf16 QK costs rel-err ~7e-3 (vs 1.8e-3 fp32) - well under the 2e-2 gate.
  - Host packs kT and qT into one [128, 2048] bf16 tensor (partitions 0-63 =
    kT d-major, 64-127 = qT d-major): one DMA stream at full 128-partition
    width, half the bytes of the old fp32 layout.
  - Scores computed transposed per (q-chunk c, k-block j): sT[k, q] =
    kq[0:64, j-block].T @ kq[64:128, q-chunk], trimmed to the causal column
    range (block granularity): col_lo = max(0, 128j - 512c). Strips are
    [128, 1024] PSUM pairs (2 k-blocks side by side).
  - Exp on ScalarE reads only the causal columns: 17408 activate-columns
    total (the hard floor; ScalarE is the bottleneck engine at ~19.5us).
  - Causal masking: one [128,128] lower-tri bf16 mask, tensor_mul on DVE over
    each of the 16 diagonal blocks after exp.
  - PV per q-block i: out[128, 66] PSUM accumulates matmul(lhsT=exp block,
    rhs=vx block) over j<=i; vx col 64 = ones -> softmax denominator.
  - Schedule interleaves chunk c's PV with chunk c+1's first QK pairs so
    ScalarE never idles at chunk boundaries.
  - Normalize: reciprocal_approx_fast + tensor_scalar_mul on DVE; one output
    DMA per chunk. Host un-permutes [128, 16*64] back to [2048, 64].
"""

import numpy as np

S = 2048
D = 64
B = 8
P = 128
CH = 512            # q-chunk width
NBLK = S // P       # 16 k-blocks
W = 66              # v | ones | pad

CORR = False        # q-split correction matmuls (2x QK rows, better accuracy)

_CACHED = {}


def _build():
    import concourse.bass as bass
    import concourse.bacc as bacc
    import concourse.mybir as mybir
    import concourse.tile as tile

    f32 = mybir.dt.float32
    bf16 = mybir.dt.bfloat16
    Exp = mybir.ActivationFunctionType.Exp

    nc = bacc.Bacc("TRN2", target_bir_lowering=False, debug=False,
                   enable_asserts=False, num_devices=B)

    assert not CORR, "CORR not supported in the [2,3,1,0] packed layout"
    kq_d = nc.dram_tensor("kq", (64, 2 * S), bf16, kind="ExternalInput")
    vx_d = nc.dram_tensor("vx", (P, NBLK * W), bf16, kind="ExternalInput")
    mask_d = nc.dram_tensor("mask", (P, P), bf16, kind="ExternalInput")
    out_d = nc.dram_tensor("out", (P, NBLK * D), f32, kind="ExternalOutput")

    with tile.TileContext(nc) as tc:
        with (
            tc.tile_pool(name="const", bufs=1) as cpool,
            tc.tile_pool(name="exps", bufs=20) as epool,
            tc.tile_pool(name="small", bufs=4) as spool,
            tc.tile_pool(name="spsum", bufs=3, space=bass.MemorySpace.PSUM) as sppool,
            tc.tile_pool(name="opsum", bufs=2, space=bass.MemorySpace.PSUM) as oppool,
        ):
            kq_s = cpool.tile([64, 2 * S], bf16, tag="kq", name="kq_s")
            vx_s = cpool.tile([P, NBLK * W], bf16, tag="vx", name="vx_s")
            mask_s = cpool.tile([P, P], bf16, tag="mask", name="mask_s")
            ostage = cpool.tile([P, NBLK * D], f32, tag="ostage", name="ostage_s")
            scr_in = cpool.tile([P, 1], f32, tag="scr_in", name="scr_in")
            scr_out = cpool.tile([P, 1], f32, tag="scr_out", name="scr_out")

            # Input DMAs in consumption order (chunk 2 first); mask/vx on
            # other engine queues so their issue overlaps.
            nc.sync.dma_start(kq_s[:, 0:2 * CH], kq_d.ap()[:, 0:2 * CH])
            nc.scalar.dma_start(mask_s[:], mask_d.ap()[:])
            nc.gpsimd.dma_start(vx_s[:], vx_d.ap()[:])

            # Warm the Exp activation table during the DMA lead-in.
            nc.gpsimd.memset(scr_in[:], 0.0)
            nc.scalar.activation(scr_out[:], scr_in[:], Exp)
            nc.sync.dma_start(kq_s[:, 2 * CH:6 * CH], kq_d.ap()[:, 2 * CH:6 * CH])
            nc.sync.dma_start(kq_s[:, 6 * CH:8 * CH], kq_d.ap()[:, 6 * CH:8 * CH])

            # Packed kq column layout: 3 windows in consumption order, each
            # [q | k] for x-ranges [0,512), [512,1536), [1536,2048).
            _w0 = [0, 1024, 3072]
            _wx = [0, 512, 1536]
            _wn = [512, 1024, 512]

            def _seg(x):
                return 0 if x < 512 else 1 if x < 1536 else 2

            def pq_hi(x):
                s = _seg(x)
                return _w0[s] + (x - _wx[s])

            def pk(x):
                s = _seg(x)
                return _w0[s] + _wn[s] + (x - _wx[s])

            eb = {}

            def qk_pair(c, p):
                """Scores + exp (+ diag mask) for k-blocks (2p, 2p+1) vs chunk c."""
                j0, j1 = 2 * p, 2 * p + 1
                sp = sppool.tile([P, 2 * CH], f32, tag="scores", name="scores")
                for t, j in enumerate((j0, j1)):
                    lo = max(0, P * j - CH * c)
                    kc = pk(j * P)
                    qh = pq_hi(c * CH + lo)
                    nc.tensor.matmul(
                        sp[:, t * CH + lo:(t + 1) * CH],
                        kq_s[:, kc:kc + P],
                        kq_s[:, qh:qh + CH - lo],
                        start=True, stop=True)
                ebt = epool.tile([P, 2 * CH], bf16, tag="eb", name="eb")
                eb[(c, p)] = ebt
                if j1 < 4 * c:
                    nc.scalar.activation(ebt[:], sp[:], Exp)
                else:
                    for t, j in enumerate((j0, j1)):
                        lo = max(0, P * j - CH * c)
                        nc.scalar.activation(
                            ebt[:, t * CH + lo:(t + 1) * CH],
                            sp[:, t * CH + lo:(t + 1) * CH], Exp)
                for t, j in enumerate((j0, j1)):
                    ii = j - 4 * c
                    if 0 <= ii < 4:
                        col = t * CH + P * ii
                        nc.vector.tensor_mul(
                            ebt[:, col:col + P], ebt[:, col:col + P], mask_s[:])

            def pv_block(c, ii, raw=False):
                """PV accumulation + normalize for q-block i = 4c + ii."""
                i = 4 * c + ii
                op = oppool.tile([P, W], f32, tag="outp", name="outp")
                for j in range(i + 1):
                    ebt = eb[(c, j // 2)]
                    col = (j % 2) * CH + ii * P
                    nc.tensor.matmul(
                        op[:], ebt[:, col:col + P], vx_s[:, j * W:(j + 1) * W],
                        start=(j == 0), stop=(j == i))
                if raw:
                    return op
                rc = spool.tile([P, 1], f32, tag="recip", name="recip")
                nc.vector.reciprocal_approx_fast(rc[:], op[:, 64:65])
                nc.vector.tensor_scalar_mul(
                    ostage[:, i * D:(i + 1) * D], op[:, 0:D], rc[:])

            # Chunk order [0,1,2,3]; chunk 3's closing pairs run [.,4,6,7,5]
            # so the PV blocks gated on pairs 6/7 overlap the last full
            # pair's exp, shrinking the post-exp drain.
            for c in range(4):
                if c == 0:
                    qk_pair(0, 0)
                    qk_pair(0, 1)
                if c < 3:
                    qk_pair(c + 1, 0)
                    qk_pair(c + 1, 1)
                    for ii in range(4):
                        pv_block(c, ii)
                    nc.sync.dma_start(
                        out_d.ap()[:, 4 * c * D:(4 * c + 4) * D],
                        ostage[:, 4 * c * D:(4 * c + 4) * D])
                    for p in range(2, 2 * (c + 1) + 2):
                        qk_pair(c + 1, p)
                else:
                    for ii in range(3):
                        pv_block(3, ii)
                    nc.sync.dma_start(
                        out_d.ap()[:, 12 * D:15 * D], ostage[:, 12 * D:15 * D])
                    pv_block(3, 3)
                    nc.sync.dma_start(
                        out_d.ap()[:, 15 * D:16 * D], ostage[:, 15 * D:16 * D])

    nc.compile()
    return nc


def get_nc():
    if "nc" not in _CACHED:
        _CACHED["nc"] = _build()
    return _CACHED["nc"]


def make_in_maps(q, k, v):
    import ml_dtypes
    bf16 = ml_dtypes.bfloat16

    q = np.asarray(q, dtype=np.float32)
    k = np.asarray(k, dtype=np.float32)
    v = np.asarray(v, dtype=np.float32)

    kl = np.arange(P)[:, None]
    ql = np.arange(P)[None, :]
    mask = (ql >= kl).astype(bf16)

    in_maps = []
    for b in range(B):
        vx = np.zeros((NBLK, P, W), dtype=bf16)
        vx[:, :, :D] = v[b].reshape(NBLK, P, D).astype(bf16)
        vx[:, :, D] = bf16(1.0)
        vx = np.ascontiguousarray(
            vx.transpose(1, 0, 2)).reshape(P, NBLK * W)
        kT = k[b].T.astype(bf16)
        qh = q[b].T.astype(bf16)
        kq = np.concatenate([
            qh[:, 0:512], kT[:, 0:512],
            qh[:, 512:1536], kT[:, 512:1536],
            qh[:, 1536:2048], kT[:, 1536:2048],
        ], axis=1)
        in_maps.append({
            "kq": np.ascontiguousarray(kq),
            "vx": vx,
            "mask": mask,
        })
    return in_maps


def kernel(q, k, v):
    from concourse.bass_utils import run_bass_kernel_spmd

    nc = get_nc()
    in_maps = make_in_maps(q, k, v)
    res = run_bass_kernel_spmd(nc, in_maps, core_ids=list(range(B)))
    _CACHED["last_results"] = res
    out = np.stack([
        res.results[b]["out"].reshape(P, NBLK, D).transpose(1, 0, 2)
        .reshape(S, D)
        for b in range(B)
    ], axis=0)
    return out.astype(np.float32)
